# revision 2
# baseline (speedup 1.0000x reference)
"""Trainium2 Bass kernel for nn_ManifoldSKI.

Model: z_{n+1} = z + gate * (tanh(sum_k a_bk (W_k z + U_k h + V_k f)) - z)
iterated to a fixed point, followed by a large vocab decode z* @ dec_w.T.

Strategy:
  - Data-parallel over batch B=512 across 8 cores (64 rows each).
  - Host prep (sharding-level): f_emb gather, h_ctx, softmax op-address
    weights `a`, weight transposes/stacking. All O(B*D + K*D^2) work.
  - Device: everything kept in transposed [D, B_local] layout so every
    matmul contracts over the partition dim with no transposes:
      * constant term cT = sum_k a_k * (U_k h + V_k f) once (22 matmuls)
      * fixed-point iterations, over-relaxed (omega) so ~12 iterations
        reach the fixed point that the reference approaches in 40.
      * decode: stream dec_w.T chunks, stationary z*T, fp32 matmuls.
  - The reference's own 40-iteration output is within 2.5e-5 (elementwise)
    of the true fixed point, so converging to the fixed point matches it.
"""

import numpy as np

B = 512
D = 128
K = 11
V = 50257
NCORES = 8
BL = B // NCORES
BETA = 5.0
ITERS = 12
OMEGA = 3.3
DEC_CHUNK = 2048  # dec_w.T free-dim chunk (1 MiB per DMA)
PSUM_N = 512      # max fp32 matmul free dim (one PSUM bank)

_built = None


def _build():
    """Build the per-core Bass module (same NEFF on all 8 cores)."""
    import concourse.bass as bass
    import concourse.mybir as mybir
    import concourse.tile as tile
    from concourse import bacc

    F32 = mybir.dt.float32
    AF = mybir.ActivationFunctionType
    ALU = mybir.AluOpType

    nc = bacc.Bacc("TRN2", target_bir_lowering=False, debug=False,
                   num_devices=NCORES)

    # DRAM I/O (per-core shards or replicated weights)
    d_hct = nc.dram_tensor("h_ctxT", [D, BL], F32, kind="ExternalInput")
    d_fet = nc.dram_tensor("f_embT", [D, BL], F32, kind="ExternalInput")
    d_zt0 = nc.dram_tensor("zT0", [D, BL], F32, kind="ExternalInput")
    d_abc = nc.dram_tensor("A_bc", [D, K * BL], F32, kind="ExternalInput")
    d_wt = nc.dram_tensor("WT", [D, K * D], F32, kind="ExternalInput")
    d_ut = nc.dram_tensor("UT", [D, K * D], F32, kind="ExternalInput")
    d_vt = nc.dram_tensor("VT", [D, K * D], F32, kind="ExternalInput")
    d_s1lo = nc.dram_tensor("s1loT", [D, D], F32, kind="ExternalInput")
    d_s1hi = nc.dram_tensor("s1hiT", [D, D], F32, kind="ExternalInput")
    d_s2t = nc.dram_tensor("s2T", [D, D], F32, kind="ExternalInput")
    d_sb1 = nc.dram_tensor("sb1c", [D, 1], F32, kind="ExternalInput")
    d_sb2 = nc.dram_tensor("sb2c", [D, 1], F32, kind="ExternalInput")
    d_ident = nc.dram_tensor("ident", [D, D], F32, kind="ExternalInput")
    d_decw = nc.dram_tensor("dec_wT", [D, V], F32, kind="ExternalInput")
    d_out = nc.dram_tensor("logits", [BL, V], F32, kind="ExternalOutput")

    with tile.TileContext(nc) as tc:
        with tc.tile_pool(name="const", bufs=1) as cp, \
             tc.tile_pool(name="work", bufs=2) as wp, \
             tc.tile_pool(name="zs", bufs=2) as zp:

            def load(dram, shape, tag):
                t = cp.tile(shape, F32, tag=tag)
                nc.sync.dma_start(t[:], dram[:])
                return t

            t_hct = load(d_hct, [D, BL], "hct")
            t_fet = load(d_fet, [D, BL], "fet")
            t_zt0 = load(d_zt0, [D, BL], "zt0")
            t_abc = load(d_abc, [D, K * BL], "abc")
            t_wt = load(d_wt, [D, K * D], "wt")
            t_ut = load(d_ut, [D, K * D], "ut")
            t_vt = load(d_vt, [D, K * D], "vt")
            t_s1lo = load(d_s1lo, [D, D], "s1lo")
            t_s1hi = load(d_s1hi, [D, D], "s1hi")
            t_s2t = load(d_s2t, [D, D], "s2t")
            t_sb1 = load(d_sb1, [D, 1], "sb1")
            t_sb2 = load(d_sb2, [D, 1], "sb2")
            t_id = load(d_ident, [D, D], "ident")

            abc3 = t_abc[:].rearrange("p (k b) -> p k b", k=K)

            def scaled(src_tile, tag):
                # X[:, k*BL + b] = A_bc[:, k*BL + b] * src[:, b]
                x = wp.tile([D, K * BL], F32, tag=tag)
                nc.vector.tensor_tensor(
                    x[:].rearrange("p (k b) -> p k b", k=K),
                    abc3,
                    src_tile[:].unsqueeze(1).broadcast_to([D, K, BL]),
                    ALU.mult,
                )
                return x

            with tc.tile_pool(name="ps", bufs=2, space="PSUM") as pp:
                # constant term cT = sum_k a_k*(U_k h + V_k f)  (transposed)
                xu = scaled(t_hct, "xu")
                xv = scaled(t_fet, "xv")
                c_ps = pp.tile([D, BL], F32, tag="pre")
                for k in range(K):
                    nc.tensor.matmul(c_ps[:], t_ut[:, k * D:(k + 1) * D],
                                     xu[:, k * BL:(k + 1) * BL],
                                     start=(k == 0), stop=False)
                for k in range(K):
                    nc.tensor.matmul(c_ps[:], t_vt[:, k * D:(k + 1) * D],
                                     xv[:, k * BL:(k + 1) * BL],
                                     start=False, stop=(k == K - 1))
                t_c = cp.tile([D, BL], F32, tag="cT")
                nc.vector.tensor_copy(t_c[:], c_ps[:])

                z_cur = t_zt0
                for it in range(ITERS):
                    xt = scaled(z_cur, "xt")
                    pre = pp.tile([D, BL], F32, tag="pre")
                    nc.tensor.matmul(pre[:], t_id[:], t_c[:],
                                     start=True, stop=False)
                    for k in range(K):
                        nc.tensor.matmul(pre[:], t_wt[:, k * D:(k + 1) * D],
                                         xt[:, k * BL:(k + 1) * BL],
                                         start=False, stop=(k == K - 1))
                    znew = zp.tile([D, BL], F32, tag="znew")
                    nc.scalar.activation(znew[:], pre[:], AF.Tanh)

                    h_ps = pp.tile([D, BL], F32, tag="h")
                    nc.tensor.matmul(h_ps[:], t_s1lo[:], z_cur[:],
                                     start=True, stop=False)
                    nc.tensor.matmul(h_ps[:], t_s1hi[:], znew[:],
                                     start=False, stop=True)
                    ht = zp.tile([D, BL], F32, tag="ht")
                    nc.scalar.activation(ht[:], h_ps[:], AF.Tanh,
                                         bias=t_sb1[:])

                    g_ps = pp.tile([D, BL], F32, tag="g")
                    nc.tensor.matmul(g_ps[:], t_s2t[:], ht[:],
                                     start=True, stop=True)
                    gt = zp.tile([D, BL], F32, tag="gt")
                    nc.scalar.activation(gt[:], g_ps[:], AF.Sigmoid,
                                         bias=t_sb2[:])

                    dd = zp.tile([D, BL], F32, tag="dd")
                    nc.vector.tensor_sub(dd[:], znew[:], z_cur[:])
                    gd = zp.tile([D, BL], F32, tag="gd")
                    # gd = (g * omega) * d   -- over-relaxed gated update
                    nc.vector.scalar_tensor_tensor(
                        gd[:], gt[:], float(OMEGA), dd[:],
                        ALU.mult, ALU.mult)
                    znext = zp.tile([D, BL], F32, tag="znext")
                    nc.vector.tensor_add(znext[:], z_cur[:], gd[:])
                    z_cur = znext

            # ---- decode: logits[b, v] = z*_b . dec_w_v ----
            with tc.tile_pool(name="dwp", bufs=3) as dwp, \
                 tc.tile_pool(name="dop", bufs=3) as dop, \
                 tc.tile_pool(name="dps", bufs=4, space="PSUM") as dps:
                nchunk = (V + DEC_CHUNK - 1) // DEC_CHUNK
                for ci in range(nchunk):
                    v0 = ci * DEC_CHUNK
                    w = min(DEC_CHUNK, V - v0)
                    wt_t = dwp.tile([D, DEC_CHUNK], F32, tag="dw")
                    nc.sync.dma_start(wt_t[:, :w], d_decw[:, v0:v0 + w])
                    ot = dop.tile([BL, DEC_CHUNK], F32, tag="do")
                    for j in range(0, w, PSUM_N):
                        jw = min(PSUM_N, w - j)
                        ps = dps.tile([BL, PSUM_N], F32, tag="dps")
                        nc.tensor.matmul(ps[:, :jw], z_cur[:],
                                         wt_t[:, j:j + jw],
                                         start=True, stop=True)
                        nc.vector.tensor_copy(ot[:, j:j + jw], ps[:, :jw])
                    nc.sync.dma_start(d_out[:, v0:v0 + w], ot[:, :w])

    nc.finalize()
    return nc


def _prep(inputs):
    """Host-side input prep + sharding. Returns (in_maps, dec_b)."""
    f = lambda name: np.ascontiguousarray(np.asarray(inputs[name]),
                                          dtype=np.float32)
    depth, complexity = f("depth"), f("complexity")
    z_init, op_emb, addr = f("z_init"), f("op_emb"), f("addr")
    W, U, Vm = f("W"), f("U"), f("Vm")
    dep_w, dep_b = f("dep_w"), f("dep_b")
    comp_w, comp_b = f("comp_w"), f("comp_b")
    sw1, sb1, sw2, sb2 = f("sw1"), f("sb1"), f("sw2"), f("sb2")
    dec_w, dec_b = f("dec_w"), f("dec_b")
    op_ids = np.asarray(inputs["op_ids"]).astype(np.int64)

    f_emb = op_emb[op_ids]                                    # [B, D]
    h_ctx = (depth @ dep_w.T + dep_b + complexity @ comp_w.T + comp_b)
    al = BETA * (h_ctx @ addr.T)                              # [B, K]
    al = al - al.max(axis=-1, keepdims=True)
    e = np.exp(al)
    a = e / e.sum(axis=-1, keepdims=True)                     # [B, K]

    c = np.ascontiguousarray
    shared = {
        "WT": c(W.transpose(2, 0, 1).reshape(D, K * D)),
        "UT": c(U.transpose(2, 0, 1).reshape(D, K * D)),
        "VT": c(Vm.transpose(2, 0, 1).reshape(D, K * D)),
        "s1loT": c(sw1[:, :D].T),
        "s1hiT": c(sw1[:, D:].T),
        "s2T": c(sw2.T),
        "sb1c": c(sb1.reshape(D, 1)),
        "sb2c": c(sb2.reshape(D, 1)),
        "ident": np.eye(D, dtype=np.float32),
        "dec_wT": c(dec_w.T),
    }
    in_maps = []
    for s in range(NCORES):
        sl = slice(s * BL, (s + 1) * BL)
        im = dict(shared)
        im["h_ctxT"] = c(h_ctx[sl].T)
        im["f_embT"] = c(f_emb[sl].T)
        im["zT0"] = c(z_init[sl].T)
        im["A_bc"] = c(np.broadcast_to(
            a[sl].T.reshape(1, K * BL), (D, K * BL)))
        in_maps.append(im)
    return in_maps, dec_b


def _run(inputs, trace=False, **kw):
    from concourse.bass_utils import run_bass_kernel_spmd
    global _built
    if _built is None:
        _built = _build()
    in_maps, dec_b = _prep(inputs)
    res = run_bass_kernel_spmd(_built, in_maps,
                               core_ids=list(range(NCORES)),
                               trace=trace, **kw)
    logits = np.concatenate([r["logits"] for r in res.results], axis=0)
    if np.any(dec_b):
        logits = logits + dec_b[None, :]
    return logits, res


def kernel(**inputs) -> np.ndarray:
    logits, _ = _run(inputs, trace=False)
    return logits


# revision 5
# speedup vs baseline: 1.5856x; 1.5856x over previous
"""Trainium2 Bass kernel for nn_ManifoldSKI.

Model: z <- z + gate * (tanh(sum_k a_bk (W_k z + U_k h + V_k f)) - z)
iterated to a fixed point, then a large vocab decode z* @ dec_w.T.

Strategy:
  - Data-parallel over batch B=512 across 8 cores (64 rows each).
  - Host prep (sharding-level, O(B*D + K*D^2)): f_emb gather, h_ctx,
    softmax op-address weights `a`, the iteration-constant term c
    (via the rank-2 structure of h_ctx and the K-entry f_emb codebook),
    weight transposes/stacking.
  - Device: transposed [D, B_local] layout throughout (no transposes):
      * fixed point: ITERS_R over-relaxed iterations with the W-matmuls
        in fp32r (single-pass, TF32-ish), then ITERS_P polish iterations
        in full fp32. The gate/stabilizer never moves the fixed point,
        so it stays fp32r always. Polish contracts fp32r error to ~1e-5.
      * decode: dec_w.T streamed in chunks; chunks are prefetched during
        the fixed point (DMA idle then); stationary z*T matmuls.
  - The reference's 40 damped iterations land within 2.5e-5 of the true
    fixed point, so converging to the fixed point reproduces it.
"""

import os
import numpy as np

B = 512
D = 128
K = 11
V = 50257
NCORES = 8
BL = B // NCORES
BETA = 5.0
ITERS_R = 7    # fp32r over-relaxed iterations
ITERS_P = 2    # fp32 polish iterations
OMEGA = 3.6
DEC_CHUNK = 2048   # dec_w.T free-dim chunk (1 MiB per DMA)
PSUM_N = 512       # max fp32 matmul free dim (one PSUM bank)
DEC_F32R = os.environ.get("DEC_F32R", "1") == "1"
PREFETCH_CHUNKS = 12  # dec_w chunks preloaded during the fixed point

_built = None


def _build():
    import concourse.bass as bass
    import concourse.mybir as mybir
    import concourse.tile as tile
    from concourse import bacc

    F32 = mybir.dt.float32
    F32R = mybir.dt.float32r
    AF = mybir.ActivationFunctionType
    ALU = mybir.AluOpType
    DEC_DT = F32R if DEC_F32R else F32

    nc = bacc.Bacc("TRN2", target_bir_lowering=False, debug=False,
                   num_devices=NCORES)

    d_zt0 = nc.dram_tensor("zT0", [D, BL], F32, kind="ExternalInput")
    d_ct = nc.dram_tensor("cT", [D, BL], F32, kind="ExternalInput")
    d_abc = nc.dram_tensor("A_bc", [D, K * BL], F32, kind="ExternalInput")
    d_wtr = nc.dram_tensor("WTr", [D, K * D], F32R, kind="ExternalInput")
    d_wt32 = nc.dram_tensor("WT32", [D, K * D], F32, kind="ExternalInput")
    d_s1lo = nc.dram_tensor("s1loT", [D, D], F32R, kind="ExternalInput")
    d_s1hi = nc.dram_tensor("s1hiT", [D, D], F32R, kind="ExternalInput")
    d_s2t = nc.dram_tensor("s2T", [D, D], F32R, kind="ExternalInput")
    d_sb1 = nc.dram_tensor("sb1c", [D, 1], F32, kind="ExternalInput")
    d_sb2 = nc.dram_tensor("sb2c", [D, 1], F32, kind="ExternalInput")
    d_ident = nc.dram_tensor("ident", [D, D], F32, kind="ExternalInput")
    d_decw = nc.dram_tensor("dec_wT", [D, V], DEC_DT, kind="ExternalInput")
    d_out = nc.dram_tensor("logits", [BL, V], F32, kind="ExternalOutput")

    with tile.TileContext(nc) as tc:
        with tc.tile_pool(name="const", bufs=1) as cp, \
             tc.tile_pool(name="work", bufs=2) as wp, \
             tc.tile_pool(name="zs", bufs=2) as zp, \
             tc.tile_pool(name="dwpre", bufs=PREFETCH_CHUNKS + 3) as dwp, \
             tc.tile_pool(name="dop", bufs=4) as dop:

            def load(dram, shape, tag, dt=F32):
                t = cp.tile(shape, dt, name="t_" + tag, tag=tag)
                nc.sync.dma_start(t[:], dram[:])
                return t

            t_zt0 = load(d_zt0, [D, BL], "zt0")
            t_c = load(d_ct, [D, BL], "cT")
            t_abc = load(d_abc, [D, K * BL], "abc")
            t_wtr = load(d_wtr, [D, K * D], "wtr", F32R)
            t_wt32 = load(d_wt32, [D, K * D], "wt32")
            t_s1lo = load(d_s1lo, [D, D], "s1lo", F32R)
            t_s1hi = load(d_s1hi, [D, D], "s1hi", F32R)
            t_s2t = load(d_s2t, [D, D], "s2t", F32R)
            t_sb1 = load(d_sb1, [D, 1], "sb1")
            t_sb2 = load(d_sb2, [D, 1], "sb2")
            t_id = load(d_ident, [D, D], "ident")

            # Prefetch the first decoder-weight chunks during the fixed
            # point: the DMA engines are otherwise idle until decode.
            nchunk = (V + DEC_CHUNK - 1) // DEC_CHUNK
            dw_tiles = []
            for ci in range(min(PREFETCH_CHUNKS, nchunk)):
                v0 = ci * DEC_CHUNK
                w = min(DEC_CHUNK, V - v0)
                t = dwp.tile([D, DEC_CHUNK], DEC_DT, name=f"dwpre{ci}",
                             tag="dw")
                nc.sync.dma_start(t[:, :w], d_decw[:, v0:v0 + w])
                dw_tiles.append(t)

            abc3 = t_abc[:].rearrange("p (k b) -> p k b", k=K)

            def scaled(src_ap, tag, dt):
                # X[:, k*BL + b] = A_bc[:, k*BL + b] * src[:, b]
                x = wp.tile([D, K * BL], dt, name="x_" + tag, tag=tag)
                nc.vector.tensor_tensor(
                    x[:].rearrange("p (k b) -> p k b", k=K),
                    abc3,
                    src_ap.unsqueeze(1).broadcast_to([D, K, BL]),
                    ALU.mult,
                )
                return x

            with tc.tile_pool(name="ps", bufs=2, space="PSUM") as pp:
                z_cur = t_zt0
                for it in range(ITERS_R + ITERS_P):
                    polish = it >= ITERS_R
                    mm_dt = F32 if polish else F32R
                    t_w = t_wt32 if polish else t_wtr
                    xt = scaled(z_cur[:], "xt32" if polish else "xtr",
                                mm_dt)
                    pre = pp.tile([D, BL], F32, name=f"pre{it}", tag="pre")
                    nc.tensor.matmul(pre[:], t_id[:], t_c[:],
                                     start=True, stop=False)
                    for k in range(K):
                        nc.tensor.matmul(pre[:], t_w[:, k * D:(k + 1) * D],
                                         xt[:, k * BL:(k + 1) * BL],
                                         start=False, stop=(k == K - 1))
                    znew = zp.tile([D, BL], F32, name=f"znew{it}",
                                   tag="znew")
                    nc.scalar.activation(znew[:], pre[:], AF.Tanh)

                    # stabilizer gate: precision here cannot move the
                    # fixed point (update is 0 at z*), so always fp32r.
                    zr = zp.tile([D, BL], F32R, name=f"zr{it}", tag="zr")
                    nc.vector.tensor_copy(zr[:], z_cur[:])
                    znr = zp.tile([D, BL], F32R, name=f"znr{it}", tag="znr")
                    nc.vector.tensor_copy(znr[:], znew[:])
                    h_ps = pp.tile([D, BL], F32, name=f"h{it}", tag="h")
                    nc.tensor.matmul(h_ps[:], t_s1lo[:], zr[:],
                                     start=True, stop=False)
                    nc.tensor.matmul(h_ps[:], t_s1hi[:], znr[:],
                                     start=False, stop=True)
                    ht = zp.tile([D, BL], F32R, name=f"ht{it}", tag="ht")
                    nc.scalar.activation(ht[:], h_ps[:], AF.Tanh,
                                         bias=t_sb1[:])
                    g_ps = pp.tile([D, BL], F32, name=f"g{it}", tag="g")
                    nc.tensor.matmul(g_ps[:], t_s2t[:], ht[:],
                                     start=True, stop=True)
                    gt = zp.tile([D, BL], F32, name=f"gt{it}", tag="gt")
                    nc.scalar.activation(gt[:], g_ps[:], AF.Sigmoid,
                                         bias=t_sb2[:])

                    dd = zp.tile([D, BL], F32, name=f"dd{it}", tag="dd")
                    nc.vector.tensor_sub(dd[:], znew[:], z_cur[:])
                    gd = zp.tile([D, BL], F32, name=f"gd{it}", tag="gd")
                    nc.vector.scalar_tensor_tensor(
                        gd[:], gt[:], float(OMEGA), dd[:],
                        ALU.mult, ALU.mult)
                    znext = zp.tile([D, BL], F32, name=f"znext{it}",
                                    tag="znext")
                    nc.vector.tensor_add(znext[:], z_cur[:], gd[:])
                    z_cur = znext

            # decode needs z* in the decode dtype
            zdec = zp.tile([D, BL], DEC_DT, name="zdec", tag="zdec")
            nc.vector.tensor_copy(zdec[:], z_cur[:])

            with tc.tile_pool(name="dps", bufs=6, space="PSUM") as dps:
                for ci in range(nchunk):
                    v0 = ci * DEC_CHUNK
                    w = min(DEC_CHUNK, V - v0)
                    if ci < len(dw_tiles):
                        wt_t = dw_tiles[ci]
                    else:
                        wt_t = dwp.tile([D, DEC_CHUNK], DEC_DT,
                                        name=f"dw{ci}", tag="dw")
                        nc.sync.dma_start(wt_t[:, :w], d_decw[:, v0:v0 + w])
                    ot = dop.tile([BL, DEC_CHUNK], F32, name=f"do{ci}",
                                  tag="do")
                    for j in range(0, w, PSUM_N):
                        jw = min(PSUM_N, w - j)
                        # fp32r matmul needs an even free-dim count; the
                        # padded column is never copied out.
                        mm_jw = jw + (jw & 1) if DEC_F32R else jw
                        ps = dps.tile([BL, PSUM_N], F32,
                                      name=f"dps{ci}_{j}", tag="dps")
                        nc.tensor.matmul(ps[:, :mm_jw], zdec[:],
                                         wt_t[:, j:j + mm_jw],
                                         start=True, stop=True)
                        if (j // PSUM_N) % 2 == 0:
                            nc.vector.tensor_copy(ot[:, j:j + jw],
                                                  ps[:, :jw])
                        else:
                            nc.scalar.copy(ot[:, j:j + jw], ps[:, :jw])
                    nc.sync.dma_start(d_out[:, v0:v0 + w], ot[:, :w])

    nc.finalize()
    return nc


def _prep(inputs):
    """Host-side input prep + sharding. Returns (in_maps, dec_b)."""
    f = lambda name: np.ascontiguousarray(np.asarray(inputs[name]),
                                          dtype=np.float32)
    depth, complexity = f("depth"), f("complexity")
    z_init, op_emb, addr = f("z_init"), f("op_emb"), f("addr")
    W, U, Vm = f("W"), f("U"), f("Vm")
    dep_w, dep_b = f("dep_w"), f("dep_b")
    comp_w, comp_b = f("comp_w"), f("comp_b")
    sw1, sb1, sw2, sb2 = f("sw1"), f("sb1"), f("sw2"), f("sb2")
    dec_w, dec_b = f("dec_w"), f("dec_b")
    op_ids = np.asarray(inputs["op_ids"]).astype(np.int64)

    h_ctx = (depth @ dep_w.T + dep_b + complexity @ comp_w.T + comp_b)
    al = BETA * (h_ctx @ addr.T)                              # [B, K]
    al = al - al.max(axis=-1, keepdims=True)
    e = np.exp(al)
    a = e / e.sum(axis=-1, keepdims=True)                     # [B, K]

    # Constant term c = sum_k a_k*(U_k h + V_k f), via the rank-2
    # structure of h_ctx and the K-entry f_emb codebook (O(K^2 D^2)).
    f64 = np.float64
    u1 = np.einsum("kij,j->ki", U.astype(f64), dep_w[:, 0].astype(f64))
    u2 = np.einsum("kij,j->ki", U.astype(f64), comp_w[:, 0].astype(f64))
    ub = np.einsum("kij,j->ki", U.astype(f64),
                   (dep_b + comp_b).astype(f64))
    vf = np.einsum("kij,mj->kmi", Vm.astype(f64), op_emb.astype(f64))
    a64 = a.astype(f64)
    c_term = (depth.astype(f64) * (a64 @ u1)
              + complexity.astype(f64) * (a64 @ u2)
              + a64 @ ub
              + np.einsum("bk,kbi->bi", a64,
                          vf[:, op_ids, :]))                  # [B, D]
    c_term = c_term.astype(np.float32)

    c = np.ascontiguousarray
    shared = {
        "WTr": c(W.transpose(2, 0, 1).reshape(D, K * D)),
        "WT32": c(W.transpose(2, 0, 1).reshape(D, K * D)),
        "s1loT": c(sw1[:, :D].T),
        "s1hiT": c(sw1[:, D:].T),
        "s2T": c(sw2.T),
        "sb1c": c(sb1.reshape(D, 1)),
        "sb2c": c(sb2.reshape(D, 1)),
        "ident": np.eye(D, dtype=np.float32),
        "dec_wT": c(dec_w.T),
    }
    in_maps = []
    for s in range(NCORES):
        sl = slice(s * BL, (s + 1) * BL)
        im = dict(shared)
        im["zT0"] = c(z_init[sl].T)
        im["cT"] = c(c_term[sl].T)
        im["A_bc"] = c(np.broadcast_to(
            a[sl].T.reshape(1, K * BL), (D, K * BL)))
        in_maps.append(im)
    return in_maps, dec_b


def _run(inputs, trace=False, **kw):
    from concourse.bass_utils import run_bass_kernel_spmd
    global _built
    if _built is None:
        _built = _build()
    in_maps, dec_b = _prep(inputs)
    res = run_bass_kernel_spmd(_built, in_maps,
                               core_ids=list(range(NCORES)),
                               trace=trace, **kw)
    logits = np.concatenate([r["logits"] for r in res.results], axis=0)
    if np.any(dec_b):
        logits = logits + dec_b[None, :]
    return logits, res


def kernel(**inputs) -> np.ndarray:
    logits, _ = _run(inputs, trace=False)
    return logits


# revision 6
# speedup vs baseline: 1.7618x; 1.1111x over previous
"""Trainium2 Bass kernel for nn_ManifoldSKI.

Model: z <- z + gate * (tanh(sum_k a_bk (W_k z + U_k h + V_k f)) - z)
iterated to a fixed point, then a large vocab decode z* @ dec_w.T.

Strategy:
  - Data-parallel over batch B=512 across 8 cores (64 rows each).
  - Host prep (sharding-level, O(B*D + K*D^2)): f_emb gather, h_ctx,
    softmax op-address weights `a`, the iteration-constant term c
    (via the rank-2 structure of h_ctx and the K-entry f_emb codebook),
    weight transposes/stacking.
  - Device: transposed [D, B_local] layout throughout (no transposes):
      * fixed point: ITERS_R over-relaxed iterations with the W-matmuls
        in fp32r (single-pass, TF32-ish), then ITERS_P polish iterations
        in full fp32. The gate/stabilizer never moves the fixed point,
        so it stays fp32r always. Polish contracts fp32r error to ~1e-5.
      * decode: dec_w.T streamed in chunks; chunks are prefetched during
        the fixed point (DMA idle then); stationary z*T matmuls.
  - The reference's 40 damped iterations land within 2.5e-5 of the true
    fixed point, so converging to the fixed point reproduces it.
"""

import os
import numpy as np

B = 512
D = 128
K = 11
V = 50257
NCORES = 8
BL = B // NCORES
BETA = 5.0
ITERS_R = 5    # fp32r over-relaxed iterations
ITERS_P = 2    # fp32 polish iterations
OMEGA = 3.6
DEC_CHUNK = 2048   # dec_w.T free-dim chunk (1 MiB per DMA)
PSUM_N = 512       # max fp32 matmul free dim (one PSUM bank)
DEC_F32R = os.environ.get("DEC_F32R", "1") == "1"
PREFETCH_CHUNKS = 10  # dec_w chunks preloaded during the fixed point

_built = None


def _build():
    import concourse.bass as bass
    import concourse.mybir as mybir
    import concourse.tile as tile
    from concourse import bacc

    F32 = mybir.dt.float32
    F32R = mybir.dt.float32r
    AF = mybir.ActivationFunctionType
    ALU = mybir.AluOpType
    DEC_DT = F32R if DEC_F32R else F32

    nc = bacc.Bacc("TRN2", target_bir_lowering=False, debug=False,
                   num_devices=NCORES)

    d_zt0 = nc.dram_tensor("zT0", [D, BL], F32, kind="ExternalInput")
    d_ct = nc.dram_tensor("cT", [D, BL], F32, kind="ExternalInput")
    d_abc = nc.dram_tensor("A_bc", [D, K * BL], F32, kind="ExternalInput")
    d_wtr = nc.dram_tensor("WTr", [D, K * D], F32R, kind="ExternalInput")
    d_wt32 = nc.dram_tensor("WT32", [D, K * D], F32, kind="ExternalInput")
    d_s1lo = nc.dram_tensor("s1loT", [D, D], F32R, kind="ExternalInput")
    d_s1hi = nc.dram_tensor("s1hiT", [D, D], F32R, kind="ExternalInput")
    d_s2t = nc.dram_tensor("s2T", [D, D], F32R, kind="ExternalInput")
    d_sb1 = nc.dram_tensor("sb1c", [D, 1], F32, kind="ExternalInput")
    d_sb2 = nc.dram_tensor("sb2c", [D, 1], F32, kind="ExternalInput")
    d_ident = nc.dram_tensor("ident", [D, D], F32, kind="ExternalInput")
    d_decw = nc.dram_tensor("dec_wT", [D, V], DEC_DT, kind="ExternalInput")
    d_out = nc.dram_tensor("logits", [BL, V], F32, kind="ExternalOutput")

    with tile.TileContext(nc) as tc:
        with tc.tile_pool(name="const", bufs=1) as cp, \
             tc.tile_pool(name="work", bufs=2) as wp, \
             tc.tile_pool(name="zs", bufs=2) as zp, \
             tc.tile_pool(name="dwpre", bufs=PREFETCH_CHUNKS + 3) as dwp, \
             tc.tile_pool(name="dop", bufs=6) as dop:

            def load(dram, shape, tag, dt=F32):
                t = cp.tile(shape, dt, name="t_" + tag, tag=tag)
                nc.sync.dma_start(t[:], dram[:])
                return t

            t_zt0 = load(d_zt0, [D, BL], "zt0")
            t_c = load(d_ct, [D, BL], "cT")
            t_abc = load(d_abc, [D, K * BL], "abc")
            t_wtr = load(d_wtr, [D, K * D], "wtr", F32R)
            t_wt32 = load(d_wt32, [D, K * D], "wt32")
            t_s1lo = load(d_s1lo, [D, D], "s1lo", F32R)
            t_s1hi = load(d_s1hi, [D, D], "s1hi", F32R)
            t_s2t = load(d_s2t, [D, D], "s2t", F32R)
            t_sb1 = load(d_sb1, [D, 1], "sb1")
            t_sb2 = load(d_sb2, [D, 1], "sb2")
            t_id = load(d_ident, [D, D], "ident")

            # Prefetch the first decoder-weight chunks during the fixed
            # point: the DMA engines are otherwise idle until decode.
            nchunk = (V + DEC_CHUNK - 1) // DEC_CHUNK
            dw_tiles = []
            for ci in range(min(PREFETCH_CHUNKS, nchunk)):
                v0 = ci * DEC_CHUNK
                w = min(DEC_CHUNK, V - v0)
                t = dwp.tile([D, DEC_CHUNK], DEC_DT, name=f"dwpre{ci}",
                             tag="dw")
                nc.gpsimd.dma_start(t[:, :w], d_decw[:, v0:v0 + w])
                dw_tiles.append(t)

            abc3 = t_abc[:].rearrange("p (k b) -> p k b", k=K)

            def scaled(src_ap, tag, dt):
                # X[:, k*BL + b] = A_bc[:, k*BL + b] * src[:, b]
                x = wp.tile([D, K * BL], dt, name="x_" + tag, tag=tag)
                nc.vector.tensor_tensor(
                    x[:].rearrange("p (k b) -> p k b", k=K),
                    abc3,
                    src_ap.unsqueeze(1).broadcast_to([D, K, BL]),
                    ALU.mult,
                )
                return x

            with tc.tile_pool(name="ps", bufs=2, space="PSUM") as pp:
                z_cur = t_zt0
                for it in range(ITERS_R + ITERS_P):
                    polish = it >= ITERS_R
                    mm_dt = F32 if polish else F32R
                    t_w = t_wt32 if polish else t_wtr
                    xt = scaled(z_cur[:], "xt32" if polish else "xtr",
                                mm_dt)
                    pre = pp.tile([D, BL], F32, name=f"pre{it}", tag="pre")
                    for k in range(K):
                        nc.tensor.matmul(pre[:], t_w[:, k * D:(k + 1) * D],
                                         xt[:, k * BL:(k + 1) * BL],
                                         start=(k == 0), stop=(k == K - 1))
                    prec = zp.tile([D, BL], F32, name=f"prec{it}",
                                   tag="prec")
                    nc.vector.tensor_add(prec[:], pre[:], t_c[:])
                    znew = zp.tile([D, BL], F32, name=f"znew{it}",
                                   tag="znew")
                    nc.scalar.activation(znew[:], prec[:], AF.Tanh)

                    # stabilizer gate: precision here cannot move the
                    # fixed point (update is 0 at z*), so always fp32r.
                    zr = zp.tile([D, BL], F32R, name=f"zr{it}", tag="zr")
                    nc.vector.tensor_copy(zr[:], z_cur[:])
                    znr = zp.tile([D, BL], F32R, name=f"znr{it}", tag="znr")
                    nc.vector.tensor_copy(znr[:], znew[:])
                    h_ps = pp.tile([D, BL], F32, name=f"h{it}", tag="h")
                    nc.tensor.matmul(h_ps[:], t_s1lo[:], zr[:],
                                     start=True, stop=False)
                    nc.tensor.matmul(h_ps[:], t_s1hi[:], znr[:],
                                     start=False, stop=True)
                    ht = zp.tile([D, BL], F32R, name=f"ht{it}", tag="ht")
                    nc.scalar.activation(ht[:], h_ps[:], AF.Tanh,
                                         bias=t_sb1[:])
                    g_ps = pp.tile([D, BL], F32, name=f"g{it}", tag="g")
                    nc.tensor.matmul(g_ps[:], t_s2t[:], ht[:],
                                     start=True, stop=True)
                    gt = zp.tile([D, BL], F32, name=f"gt{it}", tag="gt")
                    nc.scalar.activation(gt[:], g_ps[:], AF.Sigmoid,
                                         bias=t_sb2[:])

                    dd = zp.tile([D, BL], F32, name=f"dd{it}", tag="dd")
                    nc.vector.tensor_sub(dd[:], znew[:], z_cur[:])
                    gd = zp.tile([D, BL], F32, name=f"gd{it}", tag="gd")
                    nc.vector.scalar_tensor_tensor(
                        gd[:], gt[:], float(OMEGA), dd[:],
                        ALU.mult, ALU.mult)
                    znext = zp.tile([D, BL], F32, name=f"znext{it}",
                                    tag="znext")
                    nc.vector.tensor_add(znext[:], z_cur[:], gd[:])
                    z_cur = znext

            # decode needs z* in the decode dtype
            zdec = zp.tile([D, BL], DEC_DT, name="zdec", tag="zdec")
            nc.vector.tensor_copy(zdec[:], z_cur[:])

            with tc.tile_pool(name="dps", bufs=6, space="PSUM") as dps:
                for ci in range(nchunk):
                    v0 = ci * DEC_CHUNK
                    w = min(DEC_CHUNK, V - v0)
                    if ci < len(dw_tiles):
                        wt_t = dw_tiles[ci]
                    else:
                        wt_t = dwp.tile([D, DEC_CHUNK], DEC_DT,
                                        name=f"dw{ci}", tag="dw")
                        nc.sync.dma_start(wt_t[:, :w], d_decw[:, v0:v0 + w])
                    ot = dop.tile([BL, DEC_CHUNK], F32, name=f"do{ci}",
                                  tag="do")
                    for j in range(0, w, PSUM_N):
                        jw = min(PSUM_N, w - j)
                        # fp32r matmul needs an even free-dim count; the
                        # padded column is never copied out.
                        mm_jw = jw + (jw & 1) if DEC_F32R else jw
                        ps = dps.tile([BL, PSUM_N], F32,
                                      name=f"dps{ci}_{j}", tag="dps")
                        nc.tensor.matmul(ps[:, :mm_jw], zdec[:],
                                         wt_t[:, j:j + mm_jw],
                                         start=True, stop=True)
                        if (j // PSUM_N) % 2 == 0:
                            nc.vector.tensor_copy(ot[:, j:j + jw],
                                                  ps[:, :jw])
                        else:
                            nc.scalar.copy(ot[:, j:j + jw], ps[:, :jw])
                    nc.gpsimd.dma_start(d_out[:, v0:v0 + w], ot[:, :w])

    nc.finalize()
    return nc


def _prep(inputs):
    """Host-side input prep + sharding. Returns (in_maps, dec_b)."""
    f = lambda name: np.ascontiguousarray(np.asarray(inputs[name]),
                                          dtype=np.float32)
    depth, complexity = f("depth"), f("complexity")
    z_init, op_emb, addr = f("z_init"), f("op_emb"), f("addr")
    W, U, Vm = f("W"), f("U"), f("Vm")
    dep_w, dep_b = f("dep_w"), f("dep_b")
    comp_w, comp_b = f("comp_w"), f("comp_b")
    sw1, sb1, sw2, sb2 = f("sw1"), f("sb1"), f("sw2"), f("sb2")
    dec_w, dec_b = f("dec_w"), f("dec_b")
    op_ids = np.asarray(inputs["op_ids"]).astype(np.int64)

    h_ctx = (depth @ dep_w.T + dep_b + complexity @ comp_w.T + comp_b)
    al = BETA * (h_ctx @ addr.T)                              # [B, K]
    al = al - al.max(axis=-1, keepdims=True)
    e = np.exp(al)
    a = e / e.sum(axis=-1, keepdims=True)                     # [B, K]

    # Constant term c = sum_k a_k*(U_k h + V_k f), via the rank-2
    # structure of h_ctx and the K-entry f_emb codebook (O(K^2 D^2)).
    f64 = np.float64
    u1 = np.einsum("kij,j->ki", U.astype(f64), dep_w[:, 0].astype(f64))
    u2 = np.einsum("kij,j->ki", U.astype(f64), comp_w[:, 0].astype(f64))
    ub = np.einsum("kij,j->ki", U.astype(f64),
                   (dep_b + comp_b).astype(f64))
    vf = np.einsum("kij,mj->kmi", Vm.astype(f64), op_emb.astype(f64))
    a64 = a.astype(f64)
    c_term = (depth.astype(f64) * (a64 @ u1)
              + complexity.astype(f64) * (a64 @ u2)
              + a64 @ ub
              + np.einsum("bk,kbi->bi", a64,
                          vf[:, op_ids, :]))                  # [B, D]
    c_term = c_term.astype(np.float32)

    c = np.ascontiguousarray
    shared = {
        "WTr": c(W.transpose(2, 0, 1).reshape(D, K * D)),
        "WT32": c(W.transpose(2, 0, 1).reshape(D, K * D)),
        "s1loT": c(sw1[:, :D].T),
        "s1hiT": c(sw1[:, D:].T),
        "s2T": c(sw2.T),
        "sb1c": c(sb1.reshape(D, 1)),
        "sb2c": c(sb2.reshape(D, 1)),
        "ident": np.eye(D, dtype=np.float32),
        "dec_wT": c(dec_w.T),
    }
    in_maps = []
    for s in range(NCORES):
        sl = slice(s * BL, (s + 1) * BL)
        im = dict(shared)
        im["zT0"] = c(z_init[sl].T)
        im["cT"] = c(c_term[sl].T)
        im["A_bc"] = c(np.broadcast_to(
            a[sl].T.reshape(1, K * BL), (D, K * BL)))
        in_maps.append(im)
    return in_maps, dec_b


def _run(inputs, trace=False, **kw):
    from concourse.bass_utils import run_bass_kernel_spmd
    global _built
    if _built is None:
        _built = _build()
    in_maps, dec_b = _prep(inputs)
    res = run_bass_kernel_spmd(_built, in_maps,
                               core_ids=list(range(NCORES)),
                               trace=trace, **kw)
    logits = np.concatenate([r["logits"] for r in res.results], axis=0)
    if np.any(dec_b):
        logits = logits + dec_b[None, :]
    return logits, res


def kernel(**inputs) -> np.ndarray:
    logits, _ = _run(inputs, trace=False)
    return logits


# revision 7
# speedup vs baseline: 2.4781x; 1.4065x over previous
"""Trainium2 Bass kernel for nn_ManifoldSKI.

Model: z <- z + gate * (tanh(sum_k a_bk (W_k z + U_k h + V_k f)) - z)
iterated to a fixed point, then a large vocab decode z* @ dec_w.T.

Key structural facts exploited:
  - The gate cannot move the fixed point (the update vanishes at z*), and
    the ungated map z <- tanh(sum_k a_k W_k z + c) is a strong contraction
    (max_b ||sum_k a_bk W_k||_2 ~= 0.25): it converges ~10x per iteration
    from z=0. So the stabilizer network is never evaluated at all, and
    ~6 iterations reach the fixed point to ~2e-5 even in fp32r.
  - The reference's own 40 damped iterations stop within 2.5e-5 of that
    same fixed point.
  - The constant term c folds to host-side O(K^2 D^2) work via the rank-2
    structure of h_ctx and the K-entry f_emb codebook.

Layout / sharding:
  - Every core runs the (cheap) fixed point for the FULL batch B=512 in
    transposed layout zT [D, B], with the K weight matmuls in fp32r
    (single-pass, full rate at free-dim 512).
  - The big memory-bound decode is sharded over the vocab: each core owns
    a 6284-column slice of dec_w.T (loaded once at startup, ~3.2 MB) and
    writes logits[:, slice]; host concatenates. No collectives needed.
"""

import os
import numpy as np

B = 512
D = 128
K = 11
V = 50257
NCORES = 8
VS = 6284          # per-core vocab slice (8*6284 = 50272, zero-padded)
BETA = 5.0
ITERS = 6          # plain-map fp32r iterations (first is tanh(c) if z0=0)
DEC_CHUNK = 2048
PSUM_N = 512
DEC_F32R = os.environ.get("DEC_F32R", "1") == "1"

_built = {}


def _build(z0_zero):
    import concourse.bass as bass
    import concourse.mybir as mybir
    import concourse.tile as tile
    from concourse import bacc

    F32 = mybir.dt.float32
    F32R = mybir.dt.float32r
    AF = mybir.ActivationFunctionType
    ALU = mybir.AluOpType
    DEC_DT = F32R if DEC_F32R else F32

    nc = bacc.Bacc("TRN2", target_bir_lowering=False, debug=False,
                   num_devices=NCORES)

    d_ct = nc.dram_tensor("cT", [D, B], F32, kind="ExternalInput")
    d_zt0 = nc.dram_tensor("zT0", [D, B], F32, kind="ExternalInput")
    d_wtr = nc.dram_tensor("WTr", [D, K * D], F32R, kind="ExternalInput")
    d_abc = nc.dram_tensor("A_bc", [D, K * B], F32, kind="ExternalInput")
    d_decw = nc.dram_tensor("dec_wT", [D, VS], DEC_DT, kind="ExternalInput")
    d_out = nc.dram_tensor("logits", [B, VS], F32, kind="ExternalOutput")

    with tile.TileContext(nc) as tc:
        with tc.tile_pool(name="const", bufs=1) as cp, \
             tc.tile_pool(name="xts", bufs=4) as xp, \
             tc.tile_pool(name="zs", bufs=2) as zp, \
             tc.tile_pool(name="dop", bufs=6) as dop:

            # critical-path loads first (cT, WTr), then A_bc, then the
            # decoder slice (only needed after the fixed point).
            t_c = cp.tile([D, B], F32, name="t_c", tag="cT")
            nc.sync.dma_start(t_c[:], d_ct[:])
            t_wtr = cp.tile([D, K * D], F32R, name="t_wtr", tag="wtr")
            nc.sync.dma_start(t_wtr[:], d_wtr[:])
            if not z0_zero:
                t_zt0 = cp.tile([D, B], F32, name="t_zt0", tag="zt0")
                nc.sync.dma_start(t_zt0[:], d_zt0[:])
            t_abc = cp.tile([D, K * B], F32, name="t_abc", tag="abc")
            nc.gpsimd.dma_start(t_abc[:], d_abc[:])
            t_dec = cp.tile([D, VS], DEC_DT, name="t_dec", tag="dec")
            nc.gpsimd.dma_start(t_dec[:], d_decw[:])

            with tc.tile_pool(name="ps", bufs=2, space="PSUM") as pp:
                if z0_zero:
                    # z1 = tanh(c) exactly (z_init is all zeros)
                    z_cur = zp.tile([D, B], F32, name="z1", tag="z")
                    nc.scalar.activation(z_cur[:], t_c[:], AF.Tanh)
                    n_mm_iters = ITERS - 1
                else:
                    z_cur = t_zt0
                    n_mm_iters = ITERS

                for it in range(n_mm_iters):
                    pre = pp.tile([D, B], F32, name=f"pre{it}", tag="pre")
                    for k in range(K):
                        xt = xp.tile([D, B], F32R, name=f"xt{it}_{k}",
                                     tag="xt")
                        nc.vector.tensor_tensor(
                            xt[:], t_abc[:, k * B:(k + 1) * B], z_cur[:],
                            ALU.mult)
                        nc.tensor.matmul(pre[:], t_wtr[:, k * D:(k + 1) * D],
                                         xt[:], start=(k == 0),
                                         stop=(k == K - 1))
                    prec = zp.tile([D, B], F32, name=f"prec{it}",
                                   tag="prec")
                    nc.vector.tensor_add(prec[:], pre[:], t_c[:])
                    znext = zp.tile([D, B], F32, name=f"z{it + 2}", tag="z")
                    nc.scalar.activation(znext[:], prec[:], AF.Tanh)
                    z_cur = znext

            zdec = zp.tile([D, B], DEC_DT, name="zdec", tag="zdec")
            nc.vector.tensor_copy(zdec[:], z_cur[:])

            with tc.tile_pool(name="dps", bufs=6, space="PSUM") as dps:
                nchunk = (VS + DEC_CHUNK - 1) // DEC_CHUNK
                dma_i = 0
                for bi in range(B // D):           # 4 batch chunks of 128
                    zb = zdec[:, bi * D:(bi + 1) * D]
                    for ci in range(nchunk):
                        v0 = ci * DEC_CHUNK
                        w = min(DEC_CHUNK, VS - v0)
                        ot = dop.tile([D, DEC_CHUNK], F32,
                                      name=f"do{bi}_{ci}", tag="do")
                        for j in range(0, w, PSUM_N):
                            jw = min(PSUM_N, w - j)
                            mm_jw = jw + (jw & 1) if DEC_F32R else jw
                            ps = dps.tile([D, PSUM_N], F32,
                                          name=f"dps{bi}_{ci}_{j}",
                                          tag="dps")
                            nc.tensor.matmul(
                                ps[:, :mm_jw], zb,
                                t_dec[:, v0 + j:v0 + j + mm_jw],
                                start=True, stop=True)
                            if (j // PSUM_N) % 2 == 0:
                                nc.vector.tensor_copy(ot[:, j:j + jw],
                                                      ps[:, :jw])
                            else:
                                nc.scalar.copy(ot[:, j:j + jw],
                                               ps[:, :jw])
                        eng = nc.sync if dma_i % 2 == 0 else nc.gpsimd
                        dma_i += 1
                        eng.dma_start(
                            d_out[bi * D:(bi + 1) * D, v0:v0 + w],
                            ot[:, :w])

    nc.finalize()
    return nc


def _prep(inputs):
    """Host-side input prep + sharding. Returns (in_maps, dec_b, z0_zero)."""
    f = lambda name: np.ascontiguousarray(np.asarray(inputs[name]),
                                          dtype=np.float32)
    depth, complexity = f("depth"), f("complexity")
    z_init, op_emb, addr = f("z_init"), f("op_emb"), f("addr")
    W, U, Vm = f("W"), f("U"), f("Vm")
    dep_w, dep_b = f("dep_w"), f("dep_b")
    comp_w, comp_b = f("comp_w"), f("comp_b")
    dec_w, dec_b = f("dec_w"), f("dec_b")
    op_ids = np.asarray(inputs["op_ids"]).astype(np.int64)

    h_ctx = (depth @ dep_w.T + dep_b + complexity @ comp_w.T + comp_b)
    al = BETA * (h_ctx @ addr.T)                              # [B, K]
    al = al - al.max(axis=-1, keepdims=True)
    e = np.exp(al)
    a = e / e.sum(axis=-1, keepdims=True)                     # [B, K]

    # c = sum_k a_k*(U_k h + V_k f): rank-2 h_ctx + K-entry f_emb codebook
    f64 = np.float64
    u1 = np.einsum("kij,j->ki", U.astype(f64), dep_w[:, 0].astype(f64))
    u2 = np.einsum("kij,j->ki", U.astype(f64), comp_w[:, 0].astype(f64))
    ub = np.einsum("kij,j->ki", U.astype(f64),
                   (dep_b + comp_b).astype(f64))
    vf = np.einsum("kij,mj->kmi", Vm.astype(f64), op_emb.astype(f64))
    a64 = a.astype(f64)
    c_term = (depth.astype(f64) * (a64 @ u1)
              + complexity.astype(f64) * (a64 @ u2)
              + a64 @ ub
              + np.einsum("bk,kbi->bi", a64, vf[:, op_ids, :]))
    c_term = c_term.astype(np.float32)

    c = np.ascontiguousarray
    dec_wT_pad = np.zeros((D, NCORES * VS), np.float32)
    dec_wT_pad[:, :V] = dec_w.T
    shared = {
        "cT": c(c_term.T),
        "zT0": c(z_init.T),
        "WTr": c(W.transpose(2, 0, 1).reshape(D, K * D)),
        "A_bc": c(np.broadcast_to(
            a.T.reshape(1, K * B), (D, K * B))),
    }
    in_maps = []
    for s in range(NCORES):
        im = dict(shared)
        im["dec_wT"] = c(dec_wT_pad[:, s * VS:(s + 1) * VS])
        in_maps.append(im)
    z0_zero = not np.any(z_init)
    return in_maps, dec_b, z0_zero


def _run(inputs, trace=False, **kw):
    from concourse.bass_utils import run_bass_kernel_spmd
    in_maps, dec_b, z0_zero = _prep(inputs)
    if z0_zero not in _built:
        _built[z0_zero] = _build(z0_zero)
    res = run_bass_kernel_spmd(_built[z0_zero], in_maps,
                               core_ids=list(range(NCORES)),
                               trace=trace, **kw)
    logits = np.concatenate([r["logits"] for r in res.results],
                            axis=1)[:, :V]
    if np.any(dec_b):
        logits = logits + dec_b[None, :]
    return np.ascontiguousarray(logits), res


def kernel(**inputs) -> np.ndarray:
    logits, _ = _run(inputs, trace=False)
    return logits
